# revision 96
# baseline (speedup 1.0000x reference)
"""BiDAF-style attention kernel for Trainium2, data-parallel over batch on 8
cores. Shapes (hardcoded): B=16, C=2048, Q=128, E=200, O=128.

All matmuls in bf16 (1 cyc/row in the cost model, no min moving size).
Scores are computed directly in [c, q] orientation as s_cq + s_c (s_c folds
into the rhs: lhsq = w3*xqT + w1). The s_q term is applied AFTER exp as a
multiplicative per-q row: E = Etilde * exp(s_q), with exp(s_q) broadcast to
all partitions via a K=1 ones matmul; the t-matmul (S2-weighted context sum)
uses Etilde directly since per-q factors cancel in S2. z comes free from a
ones column in xcn.

S1 = softmax over q: free-dim DVE rowsum of E (r), scale, then PE-transpose
to s1t [q, c]. W2^T c2q is folded as (Xq W2^T)^T S1^T ("a2t"), so c2q is
only materialized in PSUM for the xc*c2q product and the projection needs 7
K-chunks; b_proj rides as a bias on the PSUM->SBUF out copies. The xc*c2q /
xc*q2c products are routed per chunk across DVE (direct from PSUM), ACT-
stage+DVE (bf16 2x), and ACT-stage+Pool, tuned against the timeline sim.

Emission order is tuned for the in-order engine queues: inputs(b0, b1);
ph1(b0); ph2(b0) interleaved with ph1(b1) groups (ph2 is PE-bound, the ph1
softmax chain is DVE-bound); ph2(b1) with a 2-chunk software pipeline.
"""

import numpy as np
import ml_dtypes

import concourse.bass as bass
import concourse.mybir as mybir
from concourse import bacc, masks
from concourse.bass import MemorySpace
from concourse.tile import TileContext
from concourse.bass_utils import run_bass_kernel_spmd

B, C, Q, E, O = 16, 2048, 128, 200, 128
NB = 2          # batches per core
NCORES = 8
CCH = 512       # c chunk size (psum bank)
NCH = C // CCH  # 4
CT = 128        # c tile (partitions)
NCT = C // CT   # 16
XQW = 720       # xq pack width (incl W2T chunks for a2t)
F32 = mybir.dt.float32
BF16 = mybir.dt.bfloat16
EXP = mybir.ActivationFunctionType.Exp
IDENT = mybir.ActivationFunctionType.Identity
MULT = mybir.AluOpType.mult
ADD = mybir.AluOpType.add
AXX = mybir.AxisListType.X
BF16NP = ml_dtypes.bfloat16

_CACHE = {}


def _build(num_devices=NCORES):
    nc = bacc.Bacc("TRN2", target_bir_lowering=False, debug=False,
                   num_devices=num_devices)

    d_xcta = nc.dram_tensor("xcta", [NB, 128, C], BF16, kind="ExternalInput")
    d_xctb = nc.dram_tensor("xctb", [NB, 72, C], BF16, kind="ExternalInput")
    # xcn[b, p, t, :] = [x_contexts[b, t*128+p, 0:200], 1.0, 0-pad]
    d_xcn = nc.dram_tensor("xcn", [NB, 128, NCT, 208], BF16,
                           kind="ExternalInput")
    # xq pack columns: [xqta(128) | xqtb(128, rows 0..71) | xqn(200) |
    #   w3a w1a w3b w1b w2a w2b (6) | b_proj | ones | pad]
    d_xq = nc.dram_tensor("xq", [NB, 128, XQW], BF16, kind="ExternalInput")
    # wp[p, k, :]: k0 W1T a, k1 W1T b, k2/k3 W3T a/b, k4/k5 W4T a/b,
    # k6/k7 W2T a/b (a2t source)
    d_wp = nc.dram_tensor("wp", [128, 8, O], BF16, kind="ExternalInput")
    d_out = nc.dram_tensor("out_t", [NB, O, C], BF16, kind="ExternalOutput")

    def mm(ps, lhsT, rhs, start=True, stop=True):
        nc.tensor.matmul(ps, lhsT, rhs, start=start, stop=stop)

    st = [dict() for _ in range(NB)]

    with TileContext(nc) as tc:
        with (
            tc.tile_pool(name="singles", bufs=1) as singles,
            tc.tile_pool(name="inputs", bufs=2) as inputs,
            tc.tile_pool(name="work", bufs=2) as work,
            tc.tile_pool(name="ps_big", bufs=4, space=MemorySpace.PSUM) as ps_big,
            tc.tile_pool(name="ps_o", bufs=2, space=MemorySpace.PSUM) as ps_op,
            tc.tile_pool(name="ps_tr", bufs=1, space=MemorySpace.PSUM) as ps_trp,
            tc.tile_pool(name="ps_t", bufs=1, space=MemorySpace.PSUM) as ps_tp,
        ):
            ident = singles.tile([128, 128], BF16, tag="ident")
            masks.make_identity(nc, ident)
            ones128 = singles.tile([1, 128], BF16, tag="ones128")
            nc.vector.memset(ones128, 1.0)
            wp_all = singles.tile([128, 8, O], BF16, tag="wp")

            def emit_inputs(b):
                s = st[b]
                s["xq"] = inputs.tile([128, XQW], BF16, tag="xq", name="xq%d" % b)
                nc.sync.dma_start(out=s["xq"], in_=d_xq.ap()[b])
                s["xcta"] = inputs.tile([128, C], BF16, tag="xcta", name="xcta%d" % b)
                s["xctb"] = inputs.tile([72, C], BF16, tag="xctb", name="xctb%d" % b)
                for h0, h1 in ((0, 1024), (1024, 2048)):
                    hs = slice(h0, h1)
                    nc.sync.dma_start(out=s["xcta"][:, hs],
                                      in_=d_xcta.ap()[b][:, hs])
                    nc.sync.dma_start(out=s["xctb"][:, hs],
                                      in_=d_xctb.ap()[b][:, hs])
                s["xcn"] = inputs.tile([128, NCT, 208], BF16, tag="xcn", name="xcn%d" % b)
                nc.sync.dma_start(out=s["xcn"], in_=d_xcn.ap()[b])

            def emit_ph1(b):
                s = st[b]
                xq, xcta, xctb = s["xq"], s["xcta"], s["xctb"]
                xcn = s["xcn"]
                xqta = xq[:, 0:128]
                xqtb = xq[0:72, 128:256]
                w3a, w1a = xq[:, 456:457], xq[:, 457:458]
                w3b, w1b = xq[0:72, 458:459], xq[0:72, 459:460]
                w2a, w2b = xq[:, 460:461], xq[0:72, 461:462]

                # lhsq = w3*xqT + w1 (adds s_c to the scores)
                lhsq_a = work.tile([128, Q], BF16, tag="lhsq_a")
                nc.vector.scalar_tensor_tensor(
                    lhsq_a, xqta, w3a, w1a.broadcast_to([128, Q]),
                    op0=MULT, op1=ADD)
                lhsq_b = work.tile([72, Q], BF16, tag="lhsq_b")
                nc.vector.scalar_tensor_tensor(
                    lhsq_b, xqtb, w3b, w1b.broadcast_to([72, Q]),
                    op0=MULT, op1=ADD)

                # esq_bcast[p, q] = exp(s_q[q]) on every partition:
                # s_q row -> ACT exp row -> K=1 ones matmul broadcast
                ps_sq_t = ps_op.tile([128, 512], F32, tag="o")
                ps_sq = ps_sq_t[0:1, 0:128]
                mm(ps_sq, w2a, xqta, start=True, stop=False)
                mm(ps_sq, w2b, xqtb, start=False, stop=True)
                esq_row = work.tile([1, 128], BF16, tag="esq_row")
                nc.scalar.activation(out=esq_row, in_=ps_sq, func=EXP)
                ps_eb = ps_sq_t[:, 128:256]
                mm(ps_eb, ones128, esq_row)
                esq = work.tile([128, 128], BF16, tag="esq")
                nc.vector.tensor_copy(esq, ps_eb)

                # scores -> Etilde; E = Etilde*esq; r; s1; transpose
                ecq = work.tile([128, NCT * Q], BF16, tag="ecq")
                ef = work.tile([128, NCT * Q], BF16, tag="ef")
                s1 = work.tile([128, NCT * Q], BF16, tag="s1")
                s1t = work.tile([Q, C], BF16, tag="s1t")
                rsum = work.tile([128, NCT], F32, tag="rsum")
                rcol = work.tile([128, NCT], F32, tag="rcol")
                ps_t = ps_tp.tile([Q, 256], F32, tag="t")
                s["s1t"], s["ecq"] = s1t, ecq
                for g in range(4):
                    gsl = slice(g * 512, (g + 1) * 512)
                    ps_s = ps_op.tile([128, 512], F32, tag="o")
                    for i in range(4):
                        t = 4 * g + i
                        tsl = slice(t * CT, (t + 1) * CT)
                        qs = slice(i * 128, (i + 1) * 128)
                        mm(ps_s[:, qs], xcta[:, tsl], lhsq_a,
                           start=True, stop=False)
                        mm(ps_s[:, qs], xctb[:, tsl], lhsq_b,
                           start=False, stop=True)
                    nc.scalar.activation(out=ecq[:, gsl], in_=ps_s, func=EXP)
                    # t-matmul partials (Etilde works: exp(s_q) cancels in S2)
                    for i in range(4):
                        t = 4 * g + i
                        mm(ps_t[:, 0:201], ecq[:, t * 128:(t + 1) * 128],
                           xcn[:, t, 0:201], start=(t == 0), stop=(t == 15))
                    # E = Etilde * esq (s_q enters S1 here)
                    nc.vector.tensor_mul(
                        ef[:, gsl].rearrange("p (t q) -> p t q", t=4),
                        ecq[:, gsl].rearrange("p (t q) -> p t q", t=4),
                        esq.unsqueeze(1).broadcast_to([128, 4, 128]))
                    nc.vector.tensor_reduce(
                        rsum[:, 4 * g:4 * g + 4],
                        ef[:, gsl].rearrange("p (t q) -> p t q", t=4),
                        axis=AXX, op=ADD)
                    with nc.allow_low_precision(reason="softmax scale"):
                        nc.vector.reciprocal(rcol[:, 4 * g:4 * g + 4],
                                             rsum[:, 4 * g:4 * g + 4])
                    for i in range(4):
                        t = 4 * g + i
                        ts = slice(t * 128, (t + 1) * 128)
                        nc.vector.tensor_scalar_mul(
                            s1[:, ts], ef[:, ts], rcol[:, t:t + 1])
                    ps_tr = ps_trp.tile([128, 512], BF16, tag="tr")
                    for i in range(4):
                        t = 4 * g + i
                        nc.tensor.transpose(
                            ps_tr[:, i * 128:(i + 1) * 128],
                            s1[:, t * 128:(t + 1) * 128], ident)
                    nc.vector.tensor_copy(s1t[:, gsl], ps_tr)

                # t = S2-weighted contexts [q, e]; z' in column 200
                rz = work.tile([Q, 1], F32, tag="rz")
                with nc.allow_low_precision(reason="softmax scale"):
                    nc.vector.reciprocal(rz, ps_t[:, 200:201])
                t_sb = work.tile([Q, E], BF16, tag="t_sb")
                nc.vector.tensor_scalar_mul(t_sb, ps_t[:, 0:E], rz)
                s["t_sb"] = t_sb

                # a2t[q, o] = Xq @ W2^T (emitted last: needs the wp DMA)
                ps_a2_t = ps_op.tile([128, 512], F32, tag="o")
                ps_a2 = ps_a2_t[:, 0:128]
                mm(ps_a2, xqta, xq[:, 464:592], start=True, stop=False)
                mm(ps_a2, xqtb, xq[0:72, 592:720], start=False, stop=True)
                a2t = work.tile([128, O], BF16, tag="a2t")
                nc.vector.tensor_copy(a2t, ps_a2)
                s["a2t"] = a2t

            # Product routing: "pool"/"actdve" stage the PSUM result to
            # SBUF via an ACT copy (Pool can't read PSUM; DVE gets 2x on
            # pure-bf16), "dve" multiplies straight from PSUM.
            ROUTE = {
                ("ca", 0): "actdve", ("ca", 1): "pool",
                ("ca", 2): "pool", ("ca", 3): "actdve",
                ("cb", 0): "pool", ("cb", 1): "actdve",
                ("cb", 2): "actdve", ("cb", 3): "dve",
                ("qa", 0): "actdve", ("qa", 1): "dve",
                ("qa", 2): "pool", ("qa", 3): "actdve",
                ("qb", 0): "actdve", ("qb", 1): "dve",
                ("qb", 2): "dve", ("qb", 3): "actdve",
            }

            def emit_ph2(b):
                s = st[b]
                xcta, xctb = s["xcta"], s["xctb"]
                s1t, t_sb, a2t = s["s1t"], s["t_sb"], s["a2t"]
                xqn = s["xq"][:, 256:456]
                b_col = s["xq"][:, 462:463]
                b_col32 = work.tile([128, 1], F32, tag="b_col32")
                nc.vector.tensor_copy(b_col32, b_col)
                p10 = work.tile([128, C], BF16, tag="p10")
                p11 = work.tile([72, C], BF16, tag="p11")
                p20 = work.tile([128, C], BF16, tag="p20")
                p21 = work.tile([72, C], BF16, tag="p21")
                out_sb = work.tile([O, C], BF16, tag="out_sb")

                def product(route, ps, np_, xc_sl, p_out, tag):
                    if route == "dve":
                        nc.vector.tensor_mul(p_out, ps[0:np_, :], xc_sl)
                        return
                    stage = work.tile([128, 512], BF16, tag=tag)
                    nc.scalar.copy(stage[0:np_, :], ps[0:np_, :])
                    if route == "pool":
                        nc.gpsimd.tensor_mul(p_out, stage[0:np_, :], xc_sl)
                    else:
                        nc.vector.tensor_mul(p_out, stage[0:np_, :], xc_sl)

                def cq_mms(ch):
                    sl = slice(ch * CCH, (ch + 1) * CCH)
                    ps_ca = ps_big.tile([128, 512], F32, tag="big")
                    mm(ps_ca, xqn[:, 0:128], s1t[:, sl])
                    product(ROUTE[("ca", ch)], ps_ca, 128, xcta[:, sl],
                            p10[:, sl], "stg_ca")
                    ps_cb = ps_big.tile([128, 512], F32, tag="big")
                    mm(ps_cb[0:72, :], xqn[:, 128:200], s1t[:, sl])
                    product(ROUTE[("cb", ch)], ps_cb, 72, xctb[:, sl],
                            p11[:, sl], "stg_cb")

                def q2_mms(ch):
                    sl = slice(ch * CCH, (ch + 1) * CCH)
                    ps_qa = ps_big.tile([128, 512], F32, tag="big")
                    mm(ps_qa, t_sb[:, 0:128], s1t[:, sl])
                    product(ROUTE[("qa", ch)], ps_qa, 128, xcta[:, sl],
                            p20[:, sl], "stg_qa")
                    ps_qb = ps_big.tile([128, 512], F32, tag="big")
                    mm(ps_qb[0:72, :], t_sb[:, 128:200], s1t[:, sl])
                    product(ROUTE[("qb", ch)], ps_qb, 72, xctb[:, sl],
                            p21[:, sl], "stg_qb")

                def proj(ch):
                    sl = slice(ch * CCH, (ch + 1) * CCH)
                    ps_o = ps_op.tile([128, 512], F32, tag="o")
                    mm(ps_o, wp_all[:, 0, :], xcta[:, sl],
                       start=True, stop=False)
                    mm(ps_o, wp_all[0:72, 1, :], xctb[:, sl],
                       start=False, stop=False)
                    mm(ps_o, wp_all[:, 2, :], p10[:, sl],
                       start=False, stop=False)
                    mm(ps_o, wp_all[0:72, 3, :], p11[:, sl],
                       start=False, stop=False)
                    mm(ps_o, wp_all[:, 4, :], p20[:, sl],
                       start=False, stop=False)
                    mm(ps_o, wp_all[0:72, 5, :], p21[:, sl],
                       start=False, stop=False)
                    mm(ps_o, a2t, s1t[:, sl], start=False, stop=True)
                    if ch == NCH - 1:
                        # split the last chunk so the final DMA is small
                        for h in range(2):
                            hsl = slice(ch * CCH + h * 256,
                                        ch * CCH + (h + 1) * 256)
                            hps = slice(h * 256, (h + 1) * 256)
                            nc.vector.tensor_scalar_add(
                                out_sb[:, hsl], ps_o[:, hps], b_col32)
                            nc.sync.dma_start(out=d_out.ap()[b][:, hsl],
                                              in_=out_sb[:, hsl])
                    else:
                        if ch % 2 == 0:
                            nc.scalar.activation(out=out_sb[:, sl], in_=ps_o,
                                                 func=IDENT, bias=b_col)
                        else:
                            nc.vector.tensor_scalar_add(out_sb[:, sl], ps_o,
                                                        b_col32)
                        nc.sync.dma_start(out=d_out.ap()[b][:, sl],
                                          in_=out_sb[:, sl])

                # software pipeline; the tail keeps proj(2) between the
                # last q2 matmuls and proj(3) so products can drain.
                cq_mms(0)
                cq_mms(1)
                q2_mms(0)
                proj(0)
                cq_mms(2)
                q2_mms(1)
                proj(1)
                cq_mms(3)
                q2_mms(2)
                q2_mms(3)
                proj(2)
                proj(3)

            emit_inputs(0)
            emit_inputs(1)
            nc.sync.dma_start(out=wp_all, in_=d_wp.ap())
            emit_ph1_prep(0)
            for g in range(4):
                emit_ph1_group(0, g)
            emit_ph1_tail(0)
            emit_ph1_prep(1)
            emit_ph2_prep(0)
            cq_mms(0, 0)
            emit_ph1_group(1, 0)
            cq_mms(0, 1)
            q2_mms(0, 0)
            proj(0, 0)
            emit_ph1_group(1, 1)
            cq_mms(0, 2)
            q2_mms(0, 1)
            proj(0, 1)
            emit_ph1_group(1, 2)
            cq_mms(0, 3)
            q2_mms(0, 2)
            proj(0, 2)
            emit_ph1_group(1, 3)
            q2_mms(0, 3)
            proj(0, 3)
            emit_ph1_tail(1)
            emit_ph2_prep(1)
            cq_mms(1, 0)
            cq_mms(1, 1)
            q2_mms(1, 0)
            cq_mms(1, 2)
            q2_mms(1, 1)
            proj(1, 0)
            cq_mms(1, 3)
            q2_mms(1, 2)
            proj(1, 1)
            q2_mms(1, 3)
            proj(1, 2)
            proj(1, 3)

    nc.compile()
    return nc


def _get_nc():
    if "nc" not in _CACHE:
        _CACHE["nc"] = _build()
    return _CACHE["nc"]


def kernel(x_contexts, x_questions, w_sim, w_proj, b_proj, _trace=False):
    x_contexts = np.ascontiguousarray(x_contexts, dtype=np.float32)
    x_questions = np.ascontiguousarray(x_questions, dtype=np.float32)
    w_sim = np.asarray(w_sim, dtype=np.float32)
    w_proj = np.asarray(w_proj, dtype=np.float32)
    b_proj = np.asarray(b_proj, dtype=np.float32)

    # host-side layout prep (no model math)
    xct = x_contexts.transpose(0, 2, 1)                 # [B, E, C]
    xqt = x_questions.transpose(0, 2, 1)                # [B, E, Q]
    xcta = np.ascontiguousarray(xct[:, 0:128]).astype(BF16NP)
    xctb = np.ascontiguousarray(xct[:, 128:200]).astype(BF16NP)
    xcn = np.zeros((B, 128, NCT, 208), dtype=BF16NP)
    xcn[:, :, :, 0:E] = x_contexts.reshape(B, NCT, 128, E).transpose(0, 2, 1, 3)
    xcn[:, :, :, E] = 1.0
    w1, w2, w3 = w_sim[0, 0:E], w_sim[0, E:2 * E], w_sim[0, 2 * E:]
    wpT = w_proj.T  # [800, O]
    xq = np.zeros((B, 128, XQW), dtype=BF16NP)
    xq[:, :, 0:128] = xqt[:, 0:128]
    xq[:, 0:72, 128:256] = xqt[:, 128:200]
    xq[:, :, 256:456] = x_questions
    xq[:, :, 456] = w3[0:128]
    xq[:, :, 457] = w1[0:128]
    xq[:, 0:72, 458] = w3[128:200]
    xq[:, 0:72, 459] = w1[128:200]
    xq[:, :, 460] = w2[0:128]
    xq[:, 0:72, 461] = w2[128:200]
    xq[:, :, 462] = b_proj
    xq[:, :, 463] = 1.0
    xq[:, :, 464:592] = wpT[200:328][None]
    xq[:, 0:72, 592:720] = wpT[328:400][None]
    wp = np.zeros((128, 8, O), dtype=BF16NP)
    wp[:, 0] = wpT[0:128]          # W1T a
    wp[0:72, 1] = wpT[128:200]     # W1T b
    wp[:, 2] = wpT[400:528]        # W3T a  (xc*c2q)
    wp[0:72, 3] = wpT[528:600]
    wp[:, 4] = wpT[600:728]        # W4T a  (xc*q2c)
    wp[0:72, 5] = wpT[728:800]
    wp[:, 6] = wpT[200:328]        # W2T a  (a2t source)
    wp[0:72, 7] = wpT[328:400]

    in_maps = []
    for c in range(NCORES):
        bs = slice(c * NB, (c + 1) * NB)
        in_maps.append({
            "xcta": np.ascontiguousarray(xcta[bs]),
            "xctb": np.ascontiguousarray(xctb[bs]),
            "xcn": np.ascontiguousarray(xcn[bs]),
            "xq": np.ascontiguousarray(xq[bs]),
            "wp": wp,
        })

    nc = _get_nc()
    res = run_bass_kernel_spmd(nc, in_maps, core_ids=list(range(NCORES)),
                               trace=_trace)
    _CACHE["last_res"] = res

    out = np.empty((B, C, O), np.float32)
    for c in range(NCORES):
        ot = res.results[c]["out_t"]  # [NB, O, C] bf16
        for b in range(NB):
            out[c * NB + b] = np.asarray(ot[b]).astype(np.float32).T
    return out
